# revision 3
# baseline (speedup 1.0000x reference)
"""LIF spiking-neuron scan kernel for Trainium2 (Bass/Tile), 8-core SPMD.

Reference semantics (per element, T=4 sequential steps):
    mem = 0
    for t in range(T):
        mem = mem + x[t]
        s[t] = (mem >= 1.0)          # spike, exact 0.0/1.0 fp32
        mem = mem * (mem < 1.0)      # hard reset on spike
All membrane math is fp32 and bit-exact vs the jax reference.

Sharding: x is [T*B, C, H, W] = [256, 128, 32, 32] fp32. Reshaped to
[T=4, B=64, C*H*W]; B is data-parallel sharded 8 ways. Each core's shard
is viewed as [T, 128, 8192] fp32. The T-scan is local per core.

v2 "packed" design (vs the 73-75us baseline that stored one int8 sgn per
(t, elem) = 4 MiB/core):
  - DVE: the irreducible 6-op fp32 chain per block
    (stt0, add1, stt1, add2, stt2, add3), ~56us busy.
  - act: sgn_t = Sign(u_t - 1) in bf16 {-1,0,1}, plus the final
    PSUM->int8 cast.
  - PE (idle in the baseline): packs all 4 timesteps into ONE int8 via
    4 accumulating matmuls with scaled-identity weights:
        packed = sum_t 4^t * sgn_t  in [-85, 85]
    Balanced base-4: digits in {-1,0,1} with radix 4 are uniquely
    decodable; every product 4^t*sgn_t and the <=4-term fp32 PSUM sum
    is exact. Host decodes with a 256-entry LUT (spike_t = digit_t >= 0,
    which also absorbs hw Sign(+0) returning 0 or 1).
  - Output DMA drops 4 MiB -> 1 MiB/core; DMA active ~62us -> ~53us.
Block-major emission (t innermost) keeps PSUM pack-bank lifetimes to one
block so 2 in-flight blocks fit the 8 banks.
"""

import numpy as np

import concourse.bacc as bacc
import concourse.mybir as mybir
import concourse.tile as tile
from concourse.bass_utils import run_bass_kernel_spmd

T = 4
B = 64
CHW = 128 * 32 * 32  # 131072
N_CORES = 8
B_SHARD = B // N_CORES           # 8
ELEMS = B_SHARD * CHW            # 1048576 elems per timestep per core
P = 128
F = ELEMS // P                   # 8192
MM_W = 512                       # one PSUM bank of fp32 / max moving dim

_cache = {}


def _build_pack_module(blocks=(512, 1536, 2048, 2048, 2048), x_bufs=12,
                       u_bufs=5, sgn_bufs=8, out_bufs=3, mem_bufs=2,
                       psum_bufs=2, cast_lag=1, last_store_sync=True):
    """Packed-output builder. blocks: per-block free-dim widths (sum F,
    each a multiple of 512 so matmul outputs are bank-aligned; adjacent
    pairs must fit 8 PSUM banks). cast_lag: how many of the next block's
    sign emissions precede the previous block's PSUM->int8 cast on the
    (in-order) act queue, so the cast never stalls the queue waiting on
    the tail of the pack matmuls."""
    assert sum(blocks) == F, blocks
    assert all(w % MM_W == 0 for w in blocks), blocks
    for a, b in zip(blocks, blocks[1:]):
        assert (a + b) // MM_W <= 8, (a, b)
    n_blk = len(blocks)
    col0 = [sum(blocks[:i]) for i in range(n_blk)]

    fp32 = mybir.dt.float32
    bf16 = mybir.dt.bfloat16
    int8 = mybir.dt.int8
    Alu = mybir.AluOpType

    nc = bacc.Bacc("TRN2", target_bir_lowering=False, debug=False)
    x = nc.dram_tensor("x", (T, P, F), fp32, kind="ExternalInput").ap()
    # w[:, 128*t:128*(t+1)] = 4^t * I_128 (bf16; exact small powers)
    w = nc.dram_tensor("w", (P, T * P), bf16, kind="ExternalInput").ap()
    out = nc.dram_tensor("out", (P, F), int8, kind="ExternalOutput").ap()

    with tile.TileContext(nc) as tc:
        with (
            tc.tile_pool(name="xp", bufs=x_bufs) as xpool,
            tc.tile_pool(name="up", bufs=u_bufs) as upool,
            tc.tile_pool(name="mp", bufs=mem_bufs) as mpool,
            tc.tile_pool(name="gp", bufs=sgn_bufs) as gpool,
            tc.tile_pool(name="op", bufs=out_bufs) as opool,
            tc.tile_pool(name="cp", bufs=1) as cpool,
            tc.tile_pool(name="pp", bufs=psum_bufs, space="PSUM") as ppool,
        ):
            w_sb = cpool.tile([P, T * P], bf16, tag="w", bufs=1)
            nc.sync.dma_start(out=w_sb[:], in_=w[:, :])
            neg1 = cpool.tile([P, 1], fp32, tag="neg1", bufs=1)
            nc.vector.memset(neg1[:], -1.0)

            # pending[j] = (psum tile, out tile, block slice) awaiting cast
            pending = []
            n_signs_since_pend = 0

            def flush_pending(force=False):
                nonlocal n_signs_since_pend
                if not pending:
                    return
                if force or n_signs_since_pend >= cast_lag:
                    psum_t, out_t, sl, is_last = pending.pop(0)
                    # PSUM fp32 in [-85, 85] -> int8 SBUF (exact)
                    nc.scalar.copy(out_t[:], psum_t[:])
                    q = nc.sync if (is_last and last_store_sync) else nc.scalar
                    q.dma_start(out=out[:, sl], in_=out_t[:])

            for j, wdt in enumerate(blocks):
                sl = slice(col0[j], col0[j] + wdt)
                xts = []
                for t in range(T):
                    xt = xpool.tile([P, wdt], fp32, tag="x")
                    nc.sync.dma_start(out=xt[:], in_=x[t, :, sl])
                    xts.append(xt)
                mem = mpool.tile([P, wdt], fp32, tag="mem")
                psum = ppool.tile([P, wdt], fp32, tag="pk")
                out_t = opool.tile([P, wdt], int8, tag="o")
                for t in range(T):
                    if t == 0:
                        u = xts[0]
                    else:
                        u = upool.tile([P, wdt], fp32, tag="u")
                        nc.vector.tensor_add(u[:], mem[:], xts[t][:])
                    sgn = gpool.tile([P, wdt], bf16, tag="g")
                    nc.scalar.sign(sgn[:], u[:], bias=neg1[:])
                    n_signs_since_pend += 1
                    flush_pending()
                    if t < T - 1:
                        # mem' = (u < 1) * u
                        nc.vector.scalar_tensor_tensor(
                            mem[:], u[:], 1.0, u[:], Alu.is_lt, Alu.mult)
                    # pack: psum[:, k] += 4^t * sgn[:, k]
                    for k in range(wdt // MM_W):
                        ks = slice(k * MM_W, (k + 1) * MM_W)
                        nc.tensor.matmul(
                            psum[:, ks],
                            w_sb[:, t * P:(t + 1) * P],
                            sgn[:, ks],
                            start=(t == 0),
                            stop=(t == T - 1),
                        )
                pending.append((psum, out_t, sl, j == n_blk - 1))
                n_signs_since_pend = 0
            while pending:
                flush_pending(force=True)
    nc.compile()
    return nc


def _get_module():
    if "nc" not in _cache:
        _cache["nc"] = _build_pack_module()
    return _cache["nc"]


def _pack_weights():
    # [128, 512] bf16: w[:, 128*t:128*(t+1)] = 4^t * I
    import ml_dtypes
    wt = np.zeros((P, T * P), dtype=np.float32)
    for t in range(T):
        wt[:, t * P:(t + 1) * P] = (4.0 ** t) * np.eye(P, dtype=np.float32)
    return wt.astype(ml_dtypes.bfloat16)


def _decode_lut():
    # packed = sum_t 4^t d_t, d_t in {-1,0,1}; spike_t = (d_t >= 0)
    lut = np.zeros((256, T), dtype=np.float32)
    for d0 in (-1, 0, 1):
        for d1 in (-1, 0, 1):
            for d2 in (-1, 0, 1):
                for d3 in (-1, 0, 1):
                    p = d0 + 4 * d1 + 16 * d2 + 64 * d3
                    lut[p & 0xFF] = [d0 >= 0, d1 >= 0, d2 >= 0, d3 >= 0]
    return lut


def _shard_inputs(x_np):
    # x_np: [T*B, C, H, W] fp32 -> per-core [T, P, F]
    xr = np.ascontiguousarray(x_np).reshape(T, B, CHW)
    shards = []
    for k in range(N_CORES):
        sh = np.ascontiguousarray(xr[:, k * B_SHARD : (k + 1) * B_SHARD]).reshape(
            T, P, F
        )
        shards.append(sh)
    return shards


def _unshard_outputs(outs):
    # outs: list of [P, F] int8 (balanced base-4 packed) -> [T*B,C,H,W] fp32
    lut = _decode_lut()
    full = np.empty((T, B, CHW), dtype=np.float32)
    for k, o in enumerate(outs):
        dec = lut[o.reshape(P, F).view(np.uint8)]        # [P, F, T]
        dec = np.moveaxis(dec, -1, 0).reshape(T, B_SHARD, CHW)
        full[:, k * B_SHARD : (k + 1) * B_SHARD] = dec
    return full.reshape(T * B, 128, 32, 32)


def _in_maps(x_np):
    w_np = _pack_weights()
    return [{"x": sh, "w": w_np} for sh in _shard_inputs(x_np)]


def kernel(x, T=4, **_unused):
    x_np = np.asarray(x, dtype=np.float32)
    assert int(T) == 4, f"kernel hardcoded for T=4, got {T}"
    assert x_np.shape == (256, 128, 32, 32), x_np.shape

    nc = _get_module()
    in_maps = _in_maps(x_np)
    res = run_bass_kernel_spmd(nc, in_maps, list(range(N_CORES)))
    outs = [r["out"] for r in res.results]
    return _unshard_outputs(outs)


# revision 6
# speedup vs baseline: 1.2027x; 1.2027x over previous
"""LIF spiking-neuron scan kernel for Trainium2 (Bass/Tile), 8-core SPMD.

Reference semantics (per element, T=4 sequential steps):
    mem = 0
    for t in range(T):
        mem = mem + x[t]
        s[t] = (mem >= 1.0)          # spike, exact 0.0/1.0 fp32
        mem = mem * (mem < 1.0)      # hard reset on spike
All membrane math is fp32 and bit-exact vs the jax reference.

Sharding: x is [T*B, C, H, W] = [256, 128, 32, 32] fp32. Reshaped to
[T=4, B=64, C*H*W]; B is data-parallel sharded 8 ways. Each core's shard
is viewed as [T, 128, 8192] fp32. The T-scan is local per core.

v2 "packed" design (vs the 73-75us baseline that stored one int8 sgn per
(t, elem) = 4 MiB/core):
  - DVE: the irreducible 6-op fp32 chain per block
    (stt0, add1, stt1, add2, stt2, add3), ~56us busy.
  - act: sgn_t = Sign(u_t - 1) in bf16 {-1,0,1}, plus the final
    PSUM->int8 cast.
  - PE (idle in the baseline): packs all 4 timesteps into ONE int8 via
    4 accumulating matmuls with scaled-identity weights:
        packed = sum_t 4^t * sgn_t  in [-85, 85]
    Balanced base-4: digits in {-1,0,1} with radix 4 are uniquely
    decodable; every product 4^t*sgn_t and the <=4-term fp32 PSUM sum
    is exact. Host decodes with a 256-entry LUT (spike_t = digit_t >= 0,
    which also absorbs hw Sign(+0) returning 0 or 1).
  - Output DMA drops 4 MiB -> 1 MiB/core; DMA active ~62us -> ~53us.
Block-major emission (t innermost) keeps PSUM pack-bank lifetimes to one
block so 2 in-flight blocks fit the 8 banks.
"""

import numpy as np

import concourse.bacc as bacc
import concourse.mybir as mybir
import concourse.tile as tile
from concourse.bass_utils import run_bass_kernel_spmd

T = 4
B = 64
CHW = 128 * 32 * 32  # 131072
N_CORES = 8
B_SHARD = B // N_CORES           # 8
ELEMS = B_SHARD * CHW            # 1048576 elems per timestep per core
P = 128
F = ELEMS // P                   # 8192
MM_W = 512                       # one PSUM bank of fp32 / max moving dim

_cache = {}


def _build_pack_module(blocks=(512, 2048, 2048, 2048, 1024, 512), x_bufs=12,
                       u_bufs=5, sgn_bufs=4, out_bufs=3, mem_bufs=2,
                       psum_bufs=2, cast_lag=1, last_store_sync=True):
    """Packed-output builder. blocks: per-block free-dim widths (sum F,
    each a multiple of 512 so matmul outputs are bank-aligned; adjacent
    pairs must fit 8 PSUM banks). cast_lag: how many of the next block's
    sign emissions precede the previous block's PSUM->int8 cast on the
    (in-order) act queue, so the cast never stalls the queue waiting on
    the tail of the pack matmuls.

    Spikes are stored as fp8e4 sgn pairs in [128, 2, w] tiles so the pack
    runs as DoubleRow fp8 matmuls (contraction 2x128, 0.5 cyc/row):
        psum  = [I; 4I]^T   @ [sgn0; sgn1]    (start)
        psum += [16I; 64I]^T @ [sgn2; sgn3]   (stop)
    All values ({-1,0,1} x {1,4,16,64}, sums <= 85) are exact in
    fp8e4/fp32."""
    assert sum(blocks) == F, blocks
    assert all(w % MM_W == 0 for w in blocks), blocks
    for a, b in zip(blocks, blocks[1:]):
        assert (a + b) // MM_W <= 8, (a, b)
    n_blk = len(blocks)
    col0 = [sum(blocks[:i]) for i in range(n_blk)]

    fp32 = mybir.dt.float32
    fp8 = mybir.dt.float8e4
    int8 = mybir.dt.int8
    Alu = mybir.AluOpType

    nc = bacc.Bacc("TRN2", target_bir_lowering=False, debug=False)
    x = nc.dram_tensor("x", (T, P, F), fp32, kind="ExternalInput").ap()
    # w[:, t, :] = 4^t * I_128 (fp8e4; exact small powers)
    w = nc.dram_tensor("w", (P, T, P), fp8, kind="ExternalInput").ap()
    out = nc.dram_tensor("out", (P, F), int8, kind="ExternalOutput").ap()

    with tile.TileContext(nc) as tc:
        with (
            tc.tile_pool(name="xp", bufs=x_bufs) as xpool,
            tc.tile_pool(name="up", bufs=u_bufs) as upool,
            tc.tile_pool(name="mp", bufs=mem_bufs) as mpool,
            tc.tile_pool(name="gp", bufs=sgn_bufs) as gpool,
            tc.tile_pool(name="op", bufs=out_bufs) as opool,
            tc.tile_pool(name="cp", bufs=1) as cpool,
            tc.tile_pool(name="pp", bufs=psum_bufs, space="PSUM") as ppool,
        ):
            w_sb = cpool.tile([P, T, P], fp8, tag="w", bufs=1)
            nc.sync.dma_start(out=w_sb[:], in_=w[:, :, :])
            neg1 = cpool.tile([P, 1], fp32, tag="neg1", bufs=1)
            nc.vector.memset(neg1[:], -1.0)

            # pending[j] = (psum tile, out tile, block slice) awaiting cast
            pending = []
            n_signs_since_pend = 0

            def flush_pending(force=False):
                nonlocal n_signs_since_pend
                if not pending:
                    return
                if force or n_signs_since_pend >= cast_lag:
                    psum_t, out_t, sl, is_last = pending.pop(0)
                    # PSUM fp32 in [-85, 85] -> int8 SBUF (exact)
                    nc.scalar.copy(out_t[:], psum_t[:])
                    q = nc.sync if (is_last and last_store_sync) else nc.scalar
                    q.dma_start(out=out[:, sl], in_=out_t[:])

            for j, wdt in enumerate(blocks):
                sl = slice(col0[j], col0[j] + wdt)
                xts = []
                for t in range(T):
                    xt = xpool.tile([P, wdt], fp32, tag="x")
                    nc.sync.dma_start(out=xt[:], in_=x[t, :, sl])
                    xts.append(xt)
                mem = mpool.tile([P, wdt], fp32, tag="mem")
                psum = ppool.tile([P, wdt], fp32, tag="pk")
                out_t = opool.tile([P, wdt], int8, tag="o")
                # sgn pairs: [:, 0|1, :] <- sign of (t0,t1) then (t2,t3)
                g01 = gpool.tile([P, 2, wdt], fp8, tag="g")
                g23 = gpool.tile([P, 2, wdt], fp8, tag="g")
                gpair = {0: g01, 1: g23}
                for t in range(T):
                    if t == 0:
                        u = xts[0]
                    else:
                        u = upool.tile([P, wdt], fp32, tag="u")
                        nc.vector.tensor_add(u[:], mem[:], xts[t][:])
                    sgn = gpair[t // 2][:, t % 2, :]
                    nc.scalar.sign(sgn, u[:], bias=neg1[:])
                    n_signs_since_pend += 1
                    flush_pending()
                    if t < T - 1:
                        # mem' = (u < 1) * u
                        nc.vector.scalar_tensor_tensor(
                            mem[:], u[:], 1.0, u[:], Alu.is_lt, Alu.mult)
                    if t % 2 == 1:
                        # DoubleRow pack of the finished pair
                        pair = t // 2
                        for k in range(wdt // MM_W):
                            ks = slice(k * MM_W, (k + 1) * MM_W)
                            nc.tensor.matmul(
                                psum[:, ks],
                                w_sb[:, 2 * pair:2 * pair + 2, :],
                                gpair[pair][:, :, ks],
                                start=(pair == 0),
                                stop=(pair == 1),
                                perf_mode=mybir.MatmulPerfMode.DoubleRow,
                            )
                pending.append((psum, out_t, sl, j == n_blk - 1))
                n_signs_since_pend = 0
            while pending:
                flush_pending(force=True)
    nc.compile()
    return nc


def _get_module():
    if "nc" not in _cache:
        _cache["nc"] = _build_pack_module()
    return _cache["nc"]


def _pack_weights():
    # [128, 4, 128] fp8e4: w[:, t, :] = 4^t * I
    import ml_dtypes
    wt = np.zeros((P, T, P), dtype=np.float32)
    for t in range(T):
        wt[:, t, :] = (4.0 ** t) * np.eye(P, dtype=np.float32)
    return wt.astype(ml_dtypes.float8_e4m3fn)


def _decode_lut():
    # packed = sum_t 4^t d_t, d_t in {-1,0,1}; spike_t = (d_t >= 0)
    lut = np.zeros((256, T), dtype=np.float32)
    for d0 in (-1, 0, 1):
        for d1 in (-1, 0, 1):
            for d2 in (-1, 0, 1):
                for d3 in (-1, 0, 1):
                    p = d0 + 4 * d1 + 16 * d2 + 64 * d3
                    lut[p & 0xFF] = [d0 >= 0, d1 >= 0, d2 >= 0, d3 >= 0]
    return lut


def _shard_inputs(x_np):
    # x_np: [T*B, C, H, W] fp32 -> per-core [T, P, F]
    xr = np.ascontiguousarray(x_np).reshape(T, B, CHW)
    shards = []
    for k in range(N_CORES):
        sh = np.ascontiguousarray(xr[:, k * B_SHARD : (k + 1) * B_SHARD]).reshape(
            T, P, F
        )
        shards.append(sh)
    return shards


def _unshard_outputs(outs):
    # outs: list of [P, F] int8 (balanced base-4 packed) -> [T*B,C,H,W] fp32
    lut = _decode_lut()
    full = np.empty((T, B, CHW), dtype=np.float32)
    for k, o in enumerate(outs):
        dec = lut[o.reshape(P, F).view(np.uint8)]        # [P, F, T]
        dec = np.moveaxis(dec, -1, 0).reshape(T, B_SHARD, CHW)
        full[:, k * B_SHARD : (k + 1) * B_SHARD] = dec
    return full.reshape(T * B, 128, 32, 32)


def _in_maps(x_np):
    w_np = _pack_weights()
    return [{"x": sh, "w": w_np} for sh in _shard_inputs(x_np)]


def kernel(x, T=4, **_unused):
    x_np = np.asarray(x, dtype=np.float32)
    assert int(T) == 4, f"kernel hardcoded for T=4, got {T}"
    assert x_np.shape == (256, 128, 32, 32), x_np.shape

    nc = _get_module()
    in_maps = _in_maps(x_np)
    res = run_bass_kernel_spmd(nc, in_maps, list(range(N_CORES)))
    outs = [r["out"] for r in res.results]
    return _unshard_outputs(outs)
